# revision 18
# baseline (speedup 1.0000x reference)
"""LocalRNN Trainium2 kernel: GLU -> pointwise conv -> 9-step windowed LSTM.

Full inputs in, full output out. Sharding: batch across 8 cores (2 batches/core).

v3 design notes (over v2's 372us baseline):
- Mixed-precision hidden matmuls: F/I/O gate rows in fp8e4m3 with DoubleRow
  (2 contraction rows/cycle -> 2x), G (tanh) gate rows in fp16. Measured DR
  MM out-512 = 222ns (same as fp16 N=512) with double the contraction work.
  Gate-level split keeps rel err ~6e-3 (tanh slope 4x sigmoid's => G rows
  dominate quantization error; fp8-everywhere lands 2.1e-2, over the 2e-2 bar).
- All weights pre-scaled x32 host-side (fp8 W values land in normal range);
  the G-table inject uses a 32*I stationary; ACT applies scale=1/32 for free.
- fp16 everywhere on ACT/DVE: tensor_tensor runs 2x_1P (327ns/512 cols vs
  691 fp32); h state kept in both fp16 (G matmul rhs) and fp8 (DR rhs).
- tanh(c) batched per (k,b) across the 4 j-units in one strided ACT op.
- GLU before transpose (halves transpose count; 256-col GLU ops).
- Output fp16 transposed; host does final transpose + f32 cast.
"""
from contextlib import ExitStack

import numpy as np
import ml_dtypes

import concourse.bass as bass
import concourse.mybir as mybir
import concourse.tile as tile
from concourse import bacc, bass_utils
from concourse.masks import make_identity

F32 = mybir.dt.float32
F16 = mybir.dt.float16
F8 = mybir.dt.float8e4
AF = mybir.ActivationFunctionType
DR = mybir.MatmulPerfMode.DoubleRow

N_CORES = 8
B_PER_CORE = 2          # batches per core
L = 512                 # sequence length
NT = B_PER_CORE * L     # tokens per core = 1024
D = 512                 # model dim
DH = 256                # GLU half dim
G4 = 4 * D              # 2048 gate rows
K = 9                   # window size
PAD = K - 1             # 8
LW = PAD + L            # 520: per-batch padded G row width
WSC = 32.0              # PSUM pre-scale (weights x32, ACT scale 1/32)

_cache = {}


def _build():
    nc = bacc.Bacc(
        trn_type="TRN2", target_bir_lowering=False, debug=False, num_devices=N_CORES
    )

    x_d = nc.dram_tensor("x", [NT, D], F32, kind="ExternalInput").ap()
    wf_d = nc.dram_tensor("wf", [DH, G4], F16, kind="ExternalInput").ap()
    whhg_d = nc.dram_tensor("whhg", [D, D], F16, kind="ExternalInput").ap()
    whh8_d = nc.dram_tensor("whh8", [128, 4 * 1536], F8, kind="ExternalInput").ap()
    bias_d = nc.dram_tensor("bias", [128, 32], F32, kind="ExternalInput").ap()
    out_d = nc.dram_tensor("out", [D, NT], F16, kind="ExternalOutput").ap()

    with tile.TileContext(nc) as tc, ExitStack() as top:
        const_pool = top.enter_context(tc.tile_pool(name="const", bufs=1))
        w_pool = top.enter_context(tc.tile_pool(name="weights", bufs=1))
        state_pool = top.enter_context(tc.tile_pool(name="state", bufs=1))

        ident_f32 = const_pool.tile([128, 128], F32, tag="idf")
        make_identity(nc, ident_f32[:])
        ident16 = const_pool.tile([128, 128], F16, tag="id16")
        nc.scalar.copy(ident16[:], ident_f32[:])
        ident32 = const_pool.tile([128, 128], F16, tag="id32")
        nc.scalar.mul(ident32[:], ident_f32[:], WSC)
        zeros16 = const_pool.tile([128, 16], F16, tag="z16")
        nc.gpsimd.memset(zeros16[:], 0.0)
        bias_sb = const_pool.tile([128, 32], F32, tag="bias")
        nc.sync.dma_start(bias_sb[:], bias_d)
        # pre-warm the sigmoid/tanh table set while DMAs run
        warm = const_pool.tile([128, 2], F16, tag="warm")
        nc.scalar.activation(warm[:, 0:1], ident_f32[:, 0:1], AF.Sigmoid)
        nc.scalar.activation(warm[:, 1:2], ident_f32[:, 0:1], AF.Tanh)

        wf = [w_pool.tile([128, G4], F16, tag=f"wf{ck}", name=f"wf{ck}")
              for ck in range(2)]
        whhg = [w_pool.tile([128, D], F16, tag=f"whhg{dk}", name=f"whhg{dk}")
                for dk in range(4)]
        whh8 = w_pool.tile([128, 4, 1536], F8, tag="whh8", name="whh8")

        # G table: [gate block 0..15][b*520 + col]; blocks are F,I,O,G x j
        gt = state_pool.tile([128, 16, 2 * LW], F16, tag="gt", name="gt")
        uT = [state_pool.tile([128, NT], F16, tag=f"uT{ci}", name=f"uT{ci}")
              for ci in range(2)]
        cT = state_pool.tile([128, 4, NT], F16, tag="cT", name="cT")
        H16 = [state_pool.tile([128, 4, NT], F16, tag=f"H16_{p}", name=f"H16_{p}")
               for p in range(2)]
        H8 = [state_pool.tile([128, 4, NT], F8, tag=f"H8_{p}", name=f"H8_{p}")
              for p in range(2)]

        spool = top.enter_context(tc.tile_pool(name="spool", bufs=8))
        tcp = top.enter_context(tc.tile_pool(name="tcp", bufs=3))
        tp = top.enter_context(tc.tile_pool(name="tmp", bufs=3))

        psg = top.enter_context(tc.tile_pool(name="psg", bufs=2, space="PSUM"))

        # ---------------- prep: DMA x -> GLU -> transpose -> uT ----------------
        # all input DMAs upfront, consumers-first order on the queue
        prep = ExitStack()
        xp = prep.enter_context(tc.tile_pool(name="xp", bufs=8))
        gp = prep.enter_context(tc.tile_pool(name="gp", bufs=2))

        xts = []
        for ti in range(8):
            xt = xp.tile([128, D], F32, tag="x", name="xt")
            nc.sync.dma_start(xt[:], x_d[ti * 128:(ti + 1) * 128, :])
            xts.append(xt)
            if ti == 3:
                for ck in range(2):
                    nc.sync.dma_start(wf[ck][:],
                                      wf_d[ck * 128:(ck + 1) * 128, :])
        nc.sync.dma_start(whh8[:], whh8_d.rearrange("p (s g) -> p s g", s=4))
        for dk in range(4):
            nc.sync.dma_start(whhg[dk][:], whhg_d[dk * 128:(dk + 1) * 128, :])

        def prep_chunk(ti):
            xt = xts[ti]
            sg = gp.tile([128, DH], F16, tag="sg", name="sg")
            nc.scalar.activation(sg[:], xt[:, DH:D], AF.Sigmoid)
            ut = gp.tile([128, DH], F16, tag="ut", name="ut")
            nc.vector.tensor_mul(ut[:], xt[:, 0:DH], sg[:])
            ptr = psg.tile([128, 256], F16, tag="P", name="ptr")
            for ci in range(2):
                nc.tensor.transpose(
                    ptr[:, ci * 128:(ci + 1) * 128],
                    ut[:, ci * 128:(ci + 1) * 128], ident16[:],
                )
            for ci in range(2):
                nc.vector.tensor_copy(
                    uT[ci][:, ti * 128:(ti + 1) * 128],
                    ptr[:, ci * 128:(ci + 1) * 128],
                )

        for ti in range(4):
            prep_chunk(ti)

        # pad columns: gates = bias_pad (on the otherwise-idle GPSIMD)
        for i in range(16):
            nc.gpsimd.tensor_scalar_add(
                gt[:, i, :].rearrange("p (b c) -> p b c", b=2)[:, :, 0:PAD],
                zeros16[:].rearrange("p (b c) -> p b c", b=2),
                bias_sb[:, 16 + i:16 + i + 1],
            )

        def g_phase(b):
            for j in range(4):
                P = psg.tile([128, G4], F32, tag="P", name="Pg")
                for q in range(4):
                    for ck in range(2):
                        nc.tensor.matmul(
                            P[:, q * 512:(q + 1) * 512],
                            wf[ck][:, (4 * q + j) * 128:(4 * q + j + 1) * 128],
                            uT[ck][:, b * 512:(b + 1) * 512],
                            start=(ck == 0), stop=(ck == 1),
                        )
                # split the PSUM->gt moves across DVE and ACT so the PSUM
                # buffers free faster (1 of 4 on ACT; ACT is loaded by cell0)
                for q in range(4):
                    blk = 4 * q + j
                    if q != 1:
                        nc.vector.tensor_scalar_add(
                            gt[:, blk, b * LW + PAD:b * LW + LW],
                            P[:, q * 512:(q + 1) * 512],
                            bias_sb[:, blk:blk + 1],
                        )
                    else:
                        nc.scalar.activation(
                            gt[:, blk, b * LW + PAD:b * LW + LW],
                            P[:, q * 512:(q + 1) * 512], AF.Identity,
                            bias=bias_sb[:, blk:blk + 1],
                        )

        def cell0(b):
            # step 0: c = sig(I)*tanh(G); h = sig(O)*tanh(c). Blocks 4..11 = I,O.
            # Emitted in j01/j23 halves so h0 lands incrementally and the k=1
            # units can start their early matmuls sooner.
            for half in range(2):
                j0 = 2 * half
                S0 = spool.tile([128, 4, 512], F16, tag="S", name="S0")
                nc.scalar.activation(
                    S0[:, 0:2, :],
                    gt[:, 4 + j0:6 + j0, b * LW:b * LW + 512], AF.Sigmoid)
                nc.scalar.activation(
                    S0[:, 2:4, :],
                    gt[:, 8 + j0:10 + j0, b * LW:b * LW + 512], AF.Sigmoid)
                T0 = tcp.tile([128, 2, 512], F16, tag="tc", name="T0")
                nc.scalar.activation(
                    T0[:], gt[:, 12 + j0:14 + j0, b * LW:b * LW + 512], AF.Tanh)
                for dj in range(2):
                    nc.gpsimd.tensor_mul(
                        cT[:, j0 + dj, b * 512:(b + 1) * 512], S0[:, dj, :],
                        T0[:, dj, :])
                tc0 = tcp.tile([128, 2, 512], F16, tag="tc", name="tc0")
                nc.scalar.activation(
                    tc0[:], cT[:, j0:j0 + 2, b * 512:(b + 1) * 512], AF.Tanh)
                for dj in range(2):
                    nc.gpsimd.tensor_mul(
                        H8[0][:, j0 + dj, b * 512:(b + 1) * 512],
                        S0[:, 2 + dj, :], tc0[:, dj, :])
                    nc.vector.tensor_mul(
                        H16[0][:, j0 + dj, b * 512:(b + 1) * 512],
                        S0[:, 2 + dj, :], tc0[:, dj, :])

        def unit_mm(k, b, j):
            hp = (k + 1) % 2  # buffer holding h from step k-1
            P = psg.tile([128, G4], F32, tag="P", name="P")
            for q in range(4):
                nc.tensor.matmul(
                    P[:, q * 512:(q + 1) * 512], ident32[:],
                    gt[:, 4 * q + j, b * LW + k:b * LW + k + 512],
                    start=True, stop=False,
                )

            def dr(q, s, stop):
                nc.tensor.matmul(
                    P[:, q * 512:(q + 1) * 512],
                    whh8[:, s:s + 2, (4 * q + j) * 128:(4 * q + j + 1) * 128],
                    H8[hp][:, s:s + 2, b * 512:(b + 1) * 512],
                    perf_mode=DR, start=False, stop=stop,
                    skip_group_check=True,
                )

            def gmm(dk, stop):
                nc.tensor.matmul(
                    P[:, 1536:2048],
                    whhg[dk][:, j * 128:(j + 1) * 128],
                    H16[hp][:, dk, b * 512:(b + 1) * 512],
                    start=False, stop=stop,
                )

            # consume h blocks j=0,1 first (they are produced earlier)
            for q in range(3):
                dr(q, 0, False)
            gmm(0, False)
            gmm(1, False)
            for q in range(3):
                dr(q, 2, True)
            gmm(2, False)
            gmm(3, True)
            return P

        # ---------------- emission ----------------
        g_phase(0)              # PE overlaps remaining prep chunks
        for ti in range(4, 8):
            prep_chunk(ti)
        cell0(0)                # ACT overlaps g_phase(1) PE work
        g_phase(1)
        cell0(1)
        prep.close()

        for k in range(1, K):
            for b in range(B_PER_CORE):
                hn = k % 2
                Sj = []
                tck = tcp.tile([128, 4, 512], F16, tag="tc", name="tck")

                def emit_h(j):
                    sigO = Sj[j][:, 1024:1536]
                    if k < K - 1:
                        nc.gpsimd.tensor_mul(
                            H8[hn][:, j, b * 512:(b + 1) * 512], sigO,
                            tck[:, j, :])
                    nc.vector.tensor_mul(
                        H16[hn][:, j, b * 512:(b + 1) * 512], sigO, tck[:, j, :])
                    if k == K - 1:
                        nc.sync.dma_start(
                            out_d[j * 128:(j + 1) * 128, b * 512:(b + 1) * 512],
                            H16[hn][:, j, b * 512:(b + 1) * 512],
                        )

                for j in range(4):
                    P = unit_mm(k, b, j)
                    S = spool.tile([128, G4], F16, tag="S", name="S")
                    nc.scalar.activation(
                        S[:, 0:1536], P[:, 0:1536], AF.Sigmoid, scale=1.0 / WSC)
                    nc.scalar.activation(
                        S[:, 1536:2048], P[:, 1536:2048], AF.Tanh,
                        scale=1.0 / WSC)
                    t1 = tp.tile([128, 512], F16, tag="t1", name="t1")
                    nc.vector.tensor_mul(t1[:], S[:, 512:1024], S[:, 1536:2048])
                    t2 = tp.tile([128, 512], F16, tag="t2", name="t2")
                    nc.vector.tensor_mul(
                        t2[:], S[:, 0:512], cT[:, j, b * 512:(b + 1) * 512])
                    nc.vector.tensor_add(
                        cT[:, j, b * 512:(b + 1) * 512], t1[:], t2[:])
                    Sj.append(S)
                    if j == 1:
                        # first half of tanh(c) + h writes for j=0,1
                        nc.scalar.activation(
                            tck[:, 0:2, :], cT[:, 0:2, b * 512:(b + 1) * 512],
                            AF.Tanh)
                        emit_h(0)
                        emit_h(1)
                nc.scalar.activation(
                    tck[:, 2:4, :], cT[:, 2:4, b * 512:(b + 1) * 512], AF.Tanh)
                emit_h(2)
                emit_h(3)

    nc.compile()
    return nc


def _make_in_maps(inputs):
    x = np.asarray(inputs["x"], dtype=np.float32)
    conv_w = np.asarray(inputs["conv_w"], dtype=np.float64)
    conv_b = np.asarray(inputs["conv_b"], dtype=np.float64)
    w_ih = np.asarray(inputs["w_ih"], dtype=np.float64)
    w_hh = np.asarray(inputs["w_hh"], dtype=np.float64)
    b_ih = np.asarray(inputs["b_ih"], dtype=np.float64)
    b_hh = np.asarray(inputs["b_hh"], dtype=np.float64)

    # gate permutation: torch order i,f,g,o -> F,I,O,G
    perm = np.concatenate([
        np.arange(D, 2 * D), np.arange(0, D),
        np.arange(3 * D, 4 * D), np.arange(2 * D, 3 * D),
    ])
    wf = (w_ih @ conv_w)[perm]                                  # [2048, 256]
    bias_mm = (b_ih + b_hh + w_ih @ conv_b)[perm]
    bias_pad = (b_ih + b_hh)[perm]
    whh = w_hh[perm].astype(np.float64)                         # [2048, 512]

    whhg = np.ascontiguousarray((WSC * whh[1536:2048]).T.astype(np.float16))
    w8 = (WSC * whh[0:1536]).astype(np.float32)                 # [1536, 512]
    whh8 = np.ascontiguousarray(
        w8.T.reshape(4, 128, 1536).transpose(1, 0, 2)           # [128, 4, 1536]
        .reshape(128, 4 * 1536).astype(ml_dtypes.float8_e4m3))

    bias_both = np.concatenate([
        bias_mm.astype(np.float32).reshape(16, 128).T,
        bias_pad.astype(np.float32).reshape(16, 128).T,
    ], axis=1)                                                  # [128, 32]
    shared = {
        "wf": np.ascontiguousarray(wf.T.astype(np.float16)),    # [256, 2048]
        "whhg": whhg,
        "whh8": whh8,
        "bias": np.ascontiguousarray(bias_both),
    }
    in_maps = []
    for c in range(N_CORES):
        m = dict(shared)
        m["x"] = np.ascontiguousarray(
            x[c * B_PER_CORE:(c + 1) * B_PER_CORE].reshape(NT, D)
        )
        in_maps.append(m)
    return in_maps


def kernel(x, conv_w, conv_b, w_ih, w_hh, b_ih, b_hh):
    if "nc" not in _cache:
        _cache["nc"] = _build()
    nc = _cache["nc"]

    in_maps = _make_in_maps(dict(
        x=x, conv_w=conv_w, conv_b=conv_b, w_ih=w_ih, w_hh=w_hh,
        b_ih=b_ih, b_hh=b_hh,
    ))

    res = bass_utils.run_bass_kernel_spmd(nc, in_maps, core_ids=list(range(N_CORES)))
    out = np.concatenate(
        [np.ascontiguousarray(r["out"].astype(np.float32).T)
         .reshape(B_PER_CORE, L, D) for r in res.results], axis=0
    )
    return out
